# revision 16
# baseline (speedup 1.0000x reference)
"""Causal multi-head attention (B=4, T=2048, C=1024, H=16) on 8 Trainium2 cores.

Sharding: core c handles batch b = c//2 and heads h0..h0+7 with h0 = (c%2)*8.
Each core computes QKV projection for its head slice, causal attention for its
8 heads, and a partial output projection. Host sums the two partials per batch
and adds the bias terms.

v2: fused single-pipeline, bf16 data path.
  - All matmul operands are bf16 (psum accumulation stays f32); max rel err
    ~4e-3 on the final output.  Input DMA drops 16MB -> 8MB per core.
  - The QKV projection for t-strip s+1 and the output projection for strip
    s-1 are interleaved as "filler" matmuls between the attention steps of
    strip s, so the in-order PE never idles while the Activation engine
    (exp, ~149us total) catches up, and the old serial QKV phase disappears.
  - Causal masking is a 0/1 triangle multiply on DVE after the exp (no PE
    mask matmuls), with the all-ones column in V still emitting softmax
    row-sums for free.
  - y is stored bf16 (halves output DMA); host sums partials in f32.
"""

import os
import sys
import numpy as np

sys.path.insert(0, "/opt/trn_rl_repo")

import concourse.bass as bass  # noqa: E402
import concourse.bacc as bacc  # noqa: E402
import concourse.mybir as mybir  # noqa: E402
from concourse.bass_utils import run_bass_kernel_spmd  # noqa: E402
from concourse.tile import TileContext  # noqa: E402

B, T, C, H = 4, 2048, 1024, 16
HD = C // H          # 64 head dim
HPC = 8              # heads per core
P = 128
NT = T // P          # 16 t-chunks of 128
SW = 512             # strip width (q and t strips)
NS = T // SW         # 4 strips
KC = C // P          # 8 contraction chunks for QKV
CL = HPC * HD        # 512 local channels per section
EH = HD + 1          # 65: head slot width in v (value cols + ones col)
F32 = mybir.dt.float32
BF16 = mybir.dt.bfloat16
EXPF = mybir.ActivationFunctionType.Exp
MUL = mybir.AluOpType.mult
DIV = mybir.AluOpType.divide

_CACHED = {}


def build_nc():
    nc = bacc.Bacc("TRN2", target_bir_lowering=False, debug=False)

    xt_d = nc.dram_tensor("xt", [C, T], BF16, kind="ExternalInput")
    wqk_d = nc.dram_tensor("wqk", [C, 2 * CL], BF16, kind="ExternalInput")
    wv_d = nc.dram_tensor("wv", [C, CL], BF16, kind="ExternalInput")
    wp_d = nc.dram_tensor("wp", [CL, C], BF16, kind="ExternalInput")
    bqk_d = nc.dram_tensor("bqk", [P, 8], F32, kind="ExternalInput")
    tril_d = nc.dram_tensor("tril", [P, 2 * P], BF16, kind="ExternalInput")
    ones_d = nc.dram_tensor("ones", [P, NT * HPC], BF16, kind="ExternalInput")
    y_d = nc.dram_tensor("y", [T, C], BF16, kind="ExternalOutput")

    xt_r = xt_d.ap().rearrange("(kc p) t -> p kc t", p=P)       # [128, 8, 2048]
    wqk_r = wqk_d.ap().rearrange("(kc p) c -> p kc c", p=P)     # [128, 8, 1024]
    wv_r = wv_d.ap().rearrange("(kc p) c -> p kc c", p=P)       # [128, 8, 512]
    wp_r = wp_d.ap().rearrange("(ct p) c -> p ct c", p=P)       # [128, 4, 1024]
    y_r = y_d.ap().rearrange("(tt p) c -> p tt c", p=P)         # [128, 16, 1024]

    scale = float(HD) ** -0.5
    LAG = 2

    with TileContext(nc) as tc, \
         tc.tile_pool(name="const", bufs=1) as constp, \
         tc.tile_pool(name="big", bufs=1) as bigp, \
         tc.tile_pool(name="xts", bufs=2) as xtsp, \
         tc.tile_pool(name="u_pool", bufs=6) as up, \
         tc.tile_pool(name="norm", bufs=4) as normp, \
         tc.tile_pool(name="odd", bufs=2) as oddp, \
         tc.tile_pool(name="ystage", bufs=4) as ystagep:

        qkT = bigp.tile([P, 8, T], BF16)      # c-tiles 0-3 = qT, 4-7 = kT
        v_sb = bigp.tile([P, NT, HPC * EH], BF16)
        v_heads = v_sb[:].rearrange("p t (h e) -> p t h e", e=EH)
        attnT = bigp.tile([P, 4, T], BF16)
        wqk_sb = bigp.tile([P, KC, 2 * CL], BF16)
        wv_sb = bigp.tile([P, KC, CL], BF16)
        wp_sb = bigp.tile([P, 4, C], BF16)
        bqk = constp.tile([P, 8], F32)
        tril = constp.tile([P, 2, P], BF16)

        xts_tiles = {}

        # ---------------- head: input DMAs + QKV for strip 0 --------------
        # kc-major with a dedicated 8-bank psum pool (closed before the
        # attention pools open): each 512KB chunk triple (x, wv, wqk) feeds
        # 8 matmuls as soon as it lands, so the PE tracks the DMA stream.
        xts0 = xtsp.tile([P, KC, SW], BF16, tag="xts", name="xts0")
        xts_tiles[0] = xts0
        for kc in range(KC):
            nc.sync.dma_start(xts0[:, kc, :], xt_r[:, kc, 0:SW])
            nc.sync.dma_start(wv_sb[:, kc, :], wv_r[:, kc, :])
            nc.sync.dma_start(wqk_sb[:, kc, :], wqk_r[:, kc, :])
        nc.sync.dma_start(bqk[:], bqk_d[:])
        nc.sync.dma_start(
            tril[:], tril_d.ap().rearrange("p (h q) -> p h q", h=2))
        nc.sync.dma_start(v_heads[:, :, :, HD], ones_d[:])
        nc.sync.dma_start(wp_sb[:], wp_r)

        # warm the exp table during the DMA lead-in (LoadActFuncSet is lazy
        # and otherwise lands on the first-scores critical path)
        warm = constp.tile([1, 2], F32)
        nc.gpsimd.memset(warm[:], 0.0)
        nc.scalar.activation(warm[:], warm[:], EXPF)

        # kc-major over 8 chains = 8 psum banks: the 4 v chunks plus the
        # qkT c-tiles pairs 0 and 1 need first (ct 0,4,1,5). The remaining
        # c-tiles (2,6,3,7) run as strip-0 fillers.
        HEAD_CT = (0, 4, 1, 5)
        with tc.tile_pool(name="head", bufs=8, space="PSUM") as headp:
            psv0 = [headp.tile([P, CL], F32, tag="hd", name=f"psv0_{tt}")
                    for tt in range(4)]
            psq0 = [headp.tile([P, SW], F32, tag="hd", name=f"psq0_{ct}")
                    for ct in HEAD_CT]
            for kc in range(KC):
                for tt in range(4):
                    nc.tensor.matmul(
                        psv0[tt][:],
                        xts0[:, kc, tt * P:(tt + 1) * P],
                        wv_sb[:, kc, :],
                        start=(kc == 0), stop=(kc == KC - 1),
                    )
                for i, ct in enumerate(HEAD_CT):
                    nc.tensor.matmul(
                        psq0[i][:],
                        wqk_sb[:, kc, ct * P:(ct + 1) * P],
                        xts0[:, kc, :],
                        start=(kc == 0), stop=(kc == KC - 1),
                    )
            for tt in range(4):
                nc.vector.tensor_copy(
                    v_heads[:, tt, :, 0:HD],
                    psv0[tt][:].rearrange("p (h d) -> p h d", d=HD),
                )
            for i, ct in enumerate(HEAD_CT):
                nc.vector.tensor_scalar_add(
                    qkT[:, ct, 0:SW], psq0[i][:], bqk[:, ct:ct + 1])

        with tc.tile_pool(name="mm", bufs=2, space="PSUM") as mmp, \
             tc.tile_pool(name="ps_s", bufs=2, space="PSUM") as ps_sp, \
             tc.tile_pool(name="ps_o", bufs=2, space="PSUM") as ps_op:

            # ---------------- op generators -------------------------------
            def gen_qkv(s):
                """QKV projection for strip s; one yield per instruction."""
                if s > 0:
                    t = xtsp.tile([P, KC, SW], BF16, tag="xts",
                                  name=f"xts{s}")
                    xts_tiles[s] = t
                    for kc in range(KC):
                        nc.sync.dma_start(
                            t[:, kc, :], xt_r[:, kc, s * SW:(s + 1) * SW])
                    yield
                xts = xts_tiles[s]
                for tt in range(SW // P):
                    tch = s * (SW // P) + tt
                    psv = mmp.tile([P, CL], F32, tag="mm", name="psv")
                    for kc in range(KC):
                        nc.tensor.matmul(
                            psv[:],
                            xts[:, kc, tt * P:(tt + 1) * P],
                            wv_sb[:, kc, :],
                            start=(kc == 0), stop=(kc == KC - 1),
                        )
                        yield
                    nc.vector.tensor_copy(
                        v_heads[:, tch, :, 0:HD],
                        psv[:].rearrange("p (h d) -> p h d", d=HD),
                    )
                    yield
                for ct in range(8):
                    psq = mmp.tile([P, SW], F32, tag="mm", name="psq")
                    for kc in range(KC):
                        nc.tensor.matmul(
                            psq[:],
                            wqk_sb[:, kc, ct * P:(ct + 1) * P],
                            xts[:, kc, :],
                            start=(kc == 0), stop=(kc == KC - 1),
                        )
                        yield
                    nc.vector.tensor_scalar_add(
                        qkT[:, ct, s * SW:(s + 1) * SW],
                        psq[:],
                        bqk[:, ct:ct + 1],
                    )
                    yield

            def gen_qkv0_rest():
                """qkT c-tiles 2,6,3,7 of strip 0 (chain-major), pumped as
                the first strip-0 fillers so pair pr's tiles land before
                pair pr starts."""
                for ct in (2, 6, 3, 7):
                    psq = mmp.tile([P, SW], F32, tag="mm", name="psq0r")
                    for kc in range(KC):
                        nc.tensor.matmul(
                            psq[:],
                            wqk_sb[:, kc, ct * P:(ct + 1) * P],
                            xts0[:, kc, :],
                            start=(kc == 0), stop=(kc == KC - 1),
                        )
                        yield
                    nc.vector.tensor_scalar_add(
                        qkT[:, ct, 0:SW], psq[:], bqk[:, ct:ct + 1])
                    yield

            def gen_proj(s):
                """Output projection for strip s; one yield per matmul.
                Both halves of a t-chunk share one [P, 1024] staging tile so
                each t-chunk costs a single (2KB/descriptor) y DMA."""
                for tt4 in range(SW // P):
                    tt = s * (SW // P) + tt4
                    yt = ystagep.tile([P, C], BF16, tag="yt")
                    for co in range(2):
                        psy = mmp.tile([P, 512], F32, tag="mm", name="psy")
                        for ct in range(4):
                            nc.tensor.matmul(
                                psy[:],
                                attnT[:, ct, tt * P:(tt + 1) * P],
                                wp_sb[:, ct, co * 512:(co + 1) * 512],
                                start=(ct == 0), stop=(ct == 3),
                            )
                            yield
                        nc.vector.tensor_copy(
                            yt[:, co * 512:(co + 1) * 512], psy[:])
                        yield
                    nc.sync.dma_start(y_r[:, tt, :], yt[:])

            class Pacer:
                def __init__(self, gens):
                    self.gens = list(gens)

                def pump(self, n):
                    for _ in range(n):
                        while self.gens:
                            try:
                                next(self.gens[0])
                                break
                            except StopIteration:
                                self.gens.pop(0)
                        if not self.gens:
                            return

                def drain(self):
                    while self.gens:
                        self.pump(1)

            # (boundary_pump, step_pump) per strip
            PUMPS = [(6, 8), (6, 4), (6, 3), (10, 0)]

            # ---------------- fused attention pipeline --------------------
            for s in range(NS):
                gens = []
                if s == 0:
                    gens.append(gen_qkv0_rest())
                if s < NS - 1:
                    gens.append(gen_qkv(s + 1))
                if s > 0:
                    gens.append(gen_proj(s - 1))
                pacer = Pacer(gens)
                bpump, spump = PUMPS[s]
                nk = (SW // P) * (s + 1)

                for pr in range(4):  # head pair (2pr, 2pr+1)
                    qct, kct = pr, 4 + pr
                    pacer.pump(bpump)
                    psoA = ps_op.tile([EH, SW], F32, tag="ps_o", name="psoA")
                    psoB = ps_op.tile([EH, SW], F32, tag="ps_o", name="psoB")
                    u_ring = {}
                    for step in range(nk + LAG):
                        if step < nk:
                            kt = step
                            # columns < q0 of a diagonal tile are fully
                            # masked: skip them entirely; the [128,128]
                            # block at the diagonal is masked on DVE after
                            # the exp.
                            q0 = max(0, kt * P - s * SW)
                            diag = kt >= (SW // P) * s
                            ps = ps_sp.tile([P, 2, SW], F32, tag="ps_s",
                                            name="ps")
                            u = up.tile([P, 2, SW], BF16, tag="u", name="u")
                            u_ring[kt] = u
                            for hh in range(2):
                                hp = hh * HD
                                nc.tensor.matmul(
                                    ps[:, hh, q0:SW],
                                    qkT[hp:hp + HD, kct,
                                        kt * P:(kt + 1) * P],
                                    qkT[hp:hp + HD, qct,
                                        s * SW + q0:(s + 1) * SW],
                                    start=True, stop=True,
                                )
                            nc.scalar.activation(
                                u[:, :, q0:SW], ps[:, :, q0:SW],
                                EXPF, scale=scale,
                            )
                            if diag:
                                nc.vector.tensor_tensor(
                                    u[:, :, q0:q0 + P],
                                    u[:, :, q0:q0 + P],
                                    tril[:], MUL,
                                )
                        if step >= LAG:
                            kt = step - LAG
                            u = u_ring.pop(kt)
                            q0 = max(0, kt * P - s * SW)
                            last = kt == nk - 1
                            nc.tensor.matmul(
                                psoA[0:EH, q0:SW],
                                v_sb[:, kt,
                                     (2 * pr) * EH:(2 * pr + 1) * EH],
                                u[:, 0, q0:SW],
                                start=(kt == 0), stop=last,
                            )
                            nc.tensor.matmul(
                                psoB[0:EH, q0:SW],
                                v_sb[:, kt,
                                     (2 * pr + 1) * EH:(2 * pr + 2) * EH],
                                u[:, 1, q0:SW],
                                start=(kt == 0), stop=last,
                            )
                        if step < nk:
                            pacer.pump(spump)

                    # ---- per-pair normalize ----
                    # copy psum out (frees the bank), gpsimd-broadcast the
                    # ones-row sums straight from partition 64, divide.
                    cols = slice(s * SW, (s + 1) * SW)
                    ounA = normp.tile([EH, SW], F32, tag="oun", name="ounA")
                    ounB = normp.tile([EH, SW], F32, tag="oun", name="ounB")
                    nc.vector.tensor_copy(ounA[:], psoA[:])
                    nc.vector.tensor_copy(ounB[:], psoB[:])
                    rsA = normp.tile([1, SW], F32, tag="rs", name="rsA")
                    rsB = normp.tile([1, SW], F32, tag="rs", name="rsB")
                    nc.sync.dma_start(rsA[:], ounA[HD:EH, :])
                    nc.sync.dma_start(rsB[:], ounB[HD:EH, :])
                    rcA = normp.tile([1, SW], F32, tag="rc", name="rcA")
                    rcB = normp.tile([1, SW], F32, tag="rc", name="rcB")
                    nc.vector.reciprocal(rcA[:], rsA[:])
                    nc.vector.reciprocal(rcB[:], rsB[:])
                    bcA = normp.tile([HD, SW], F32, tag="bc", name="bcA")
                    bcB = normp.tile([HD, SW], F32, tag="bc", name="bcB")
                    nc.gpsimd.partition_broadcast(bcA[:], rcA[:])
                    nc.gpsimd.partition_broadcast(bcB[:], rcB[:])
                    nc.vector.tensor_tensor(
                        attnT[0:HD, pr, cols], ounA[0:HD, :], bcA[:], MUL)
                    odd = oddp.tile([HD, SW], BF16, tag="odd", name="odd")
                    nc.vector.tensor_tensor(
                        odd[:], ounB[0:HD, :], bcB[:], MUL)
                    nc.sync.dma_start(attnT[HD:P, pr, cols], odd[:])

                pacer.drain()

        # ---------------- tail: strip 3 projection ------------------------
        # attention psum pools are closed; use all 8 banks, stage-major so
        # each chain's first three accumulation steps overlap the final
        # normalize chain.
        with tc.tile_pool(name="ps_f", bufs=8, space="PSUM") as ps_f:
            chains = []
            for tt in range(4 * (NS - 1), 4 * NS):
                for co in range(2):
                    chains.append(
                        (tt, co,
                         ps_f.tile([P, 512], F32, tag="psf",
                                   name=f"psf_{tt}_{co}")))
            for ct in range(4):
                for tt, co, psy in chains:
                    nc.tensor.matmul(
                        psy[:],
                        attnT[:, ct, tt * P:(tt + 1) * P],
                        wp_sb[:, ct, co * 512:(co + 1) * 512],
                        start=(ct == 0), stop=(ct == 3),
                    )
            yts = {}
            for i, (tt, co, psy) in enumerate(chains):
                if tt not in yts:
                    yts[tt] = ystagep.tile([P, C], BF16, tag="yt",
                                           name=f"ytf_{tt}")
                yt = yts[tt]
                if i % 2 == 0:
                    nc.scalar.copy(yt[:, co * 512:(co + 1) * 512], psy[:])
                else:
                    nc.vector.tensor_copy(
                        yt[:, co * 512:(co + 1) * 512], psy[:])
                if co == 1:
                    nc.sync.dma_start(y_r[:, tt, :], yt[:])
    nc.compile()
    return nc


def _host_consts():
    import ml_dtypes
    i_idx = np.arange(P, dtype=np.float32)[:, None]
    j_idx = np.arange(P, dtype=np.float32)[None, :]
    tr = (j_idx - i_idx >= 0).astype(ml_dtypes.bfloat16)  # [k, q]: keep k<=q
    tril = np.concatenate([tr, tr], axis=1)               # [P, 2*P]
    ones = np.ones((P, NT * HPC), dtype=ml_dtypes.bfloat16)
    return tril, ones


def make_in_maps(x, w_attn, b_attn, w_proj):
    import ml_dtypes
    bf = ml_dtypes.bfloat16
    tril, ones = _host_consts()
    in_maps = []
    for c in range(8):
        b = c // 2
        h0 = (c % 2) * HPC
        qcols = slice(h0 * HD, h0 * HD + CL)
        kcols = slice(C + h0 * HD, C + h0 * HD + CL)
        vcols = slice(2 * C + h0 * HD, 2 * C + h0 * HD + CL)
        wqk = np.concatenate([w_attn[:, qcols], w_attn[:, kcols]], axis=1)
        bqk = np.concatenate([b_attn[qcols], b_attn[kcols]]).reshape(8, P).T
        in_maps.append({
            "xt": np.ascontiguousarray(x[b].T).astype(bf),
            "wqk": np.ascontiguousarray(wqk).astype(bf),
            "wv": np.ascontiguousarray(w_attn[:, vcols]).astype(bf),
            "wp": np.ascontiguousarray(
                w_proj[h0 * HD:h0 * HD + CL, :]).astype(bf),
            "bqk": np.ascontiguousarray(bqk),
            "tril": tril,
            "ones": ones,
        })
    return in_maps


def _get_runner():
    """Build the SPMD executor once: a cached jax.jit over 8 cores.

    Mirrors bass2jax.run_bass_via_pjrt but hoists the jit so repeated
    kernel() calls reuse the compiled executable.
    """
    if "runner" in _CACHED:
        return _CACHED["runner"]
    import jax
    import jax.numpy as jnp
    from jax.sharding import Mesh, PartitionSpec
    from jax.experimental.shard_map import shard_map
    from concourse import bass2jax
    import concourse.mybir as mybir_

    nc = _CACHED.get("nc")
    if nc is None:
        nc = _CACHED["nc"] = build_nc()
    bass2jax.install_neuronx_cc_hook()

    partition_name = (nc.partition_id_tensor.name
                      if nc.partition_id_tensor else None)
    in_names, out_names, out_avals, zero_shapes = [], [], [], []
    for alloc in nc.m.functions[0].allocations:
        if not isinstance(alloc, mybir_.MemoryLocationSet):
            continue
        name = alloc.memorylocations[0].name
        if alloc.kind == "ExternalInput":
            if name != partition_name:
                in_names.append(name)
        elif alloc.kind == "ExternalOutput":
            shape = tuple(alloc.tensor_shape)
            dtype = mybir_.dt.np(alloc.dtype)
            out_names.append(name)
            out_avals.append(jax.core.ShapedArray(shape, dtype))
            zero_shapes.append((shape, dtype))
    n_params = len(in_names)
    n_outs = len(out_names)
    all_names = in_names + out_names
    if partition_name is not None:
        all_names = all_names + [partition_name]

    def _body(*args):
        operands = list(args)
        if partition_name is not None:
            operands.append(bass2jax.partition_id_tensor())
        outs = bass2jax._bass_exec_p.bind(
            *operands,
            out_avals=tuple(out_avals),
            in_names=tuple(all_names),
            out_names=tuple(out_names),
            lowering_input_output_aliases=(),
            sim_require_finite=True,
            sim_require_nnan=True,
            nc=nc,
        )
        return tuple(outs)

    devices = jax.devices()[:8]
    mesh = Mesh(np.asarray(devices), ("core",))
    in_specs = (PartitionSpec("core"),) * (n_params + n_outs)
    out_specs = (PartitionSpec("core"),) * n_outs
    donate = tuple(range(n_params, n_params + n_outs))
    sharded = jax.jit(
        shard_map(_body, mesh=mesh, in_specs=in_specs, out_specs=out_specs,
                  check_rep=False),
        donate_argnums=donate, keep_unused=True,
    )

    def run(in_maps):
        concat_in = [
            np.concatenate([np.asarray(in_maps[c][nm]) for c in range(8)],
                           axis=0)
            for nm in in_names
        ]
        concat_zeros = [
            np.zeros((8 * s[0], *s[1:]), dt) for (s, dt) in zero_shapes
        ]
        out_arrs = sharded(*concat_in, *concat_zeros)
        return [
            {nm: np.asarray(out_arrs[i]).reshape(8, *out_avals[i].shape)[c]
             for i, nm in enumerate(out_names)}
            for c in range(8)
        ]

    _CACHED["runner"] = run
    return run


def kernel(x, w_attn, b_attn, w_proj, b_proj):
    x = np.asarray(x, dtype=np.float32)
    w_attn = np.asarray(w_attn, dtype=np.float32)
    b_attn = np.asarray(b_attn, dtype=np.float32)
    w_proj = np.asarray(w_proj, dtype=np.float32)
    b_proj = np.asarray(b_proj, dtype=np.float32)

    in_maps = make_in_maps(x, w_attn, b_attn, w_proj)
    try:
        run = _get_runner()
        results = run(in_maps)
    except Exception:
        # fallback: the stock SPMD runner (slower per call, same result)
        if "nc" not in _CACHED:
            _CACHED["nc"] = build_nc()
        res = run_bass_kernel_spmd(
            _CACHED["nc"], in_maps, core_ids=list(range(8)))
        results = res.results

    # v-bias contribution: probs rows sum to 1, so attn += 1 * b_v^T, and
    # (1 b_v^T) @ w_proj = row vector b_v @ w_proj added to every position.
    extra = b_attn[2 * C:] @ w_proj + b_proj  # [C]
    out = np.empty((B, T, C), dtype=np.float32)
    for b in range(B):
        out[b] = (results[2 * b]["y"].astype(np.float32)
                  + results[2 * b + 1]["y"].astype(np.float32) + extra)
    return out
